# revision 53
# baseline (speedup 1.0000x reference)
"""Trainium2 Bass kernel for a 2-layer LSTM + dense head (batch-sharded over 8 cores).

Reference computation (PyTorch gate order i,f,g,o):
  h1 = LSTM(x;   w_ih1, w_hh1, b_ih1+b_hh1)   # D=128 -> H1=128
  h2 = LSTM(h1;  w_ih2, w_hh2, b_ih2+b_hh2)   # H1=128 -> H2=64
  out = relu(h2[:, -1] @ w_dense.T + b_dense) # [B, 64]

Device-side design (per core, B_c = 256 batch columns):
  - All state "transposed": hidden-dim on SBUF partitions, batch on free dim.
  - fp16 everywhere in SBUF (x, weights, states, gate outputs); fp32 in PSUM.
    fp16 matmuls run at 1 cycle/row; fp16 doubles DVE tensor_tensor rate.
  - Gates i,f,o use Sigmoid directly; the g gate's tanh is computed as
    tanh(z) = 2*sigmoid(2z) - 1 (g weights doubled on host), so ALL four
    gates go through a single Sigmoid ACT op per layer, and every
    elementwise op is a 2-input tensor_tensor (fp16 2x on DVE):
      sg    = sigmoid(psum[i | f | o | 2g])   # one ACT op [128,1024]
      gt    = 2*sg_g - 1                      # tensor_scalar = tanh(g)
      v     = sg_f * c ;  u = sg_i * gt ;  c' = u + v
      thc   = tanh(c') ;  h' = sg_o * thc
    (Sigmoid, Tanh, Relu all live in one HW activation table -> no reloads.)
  - Layer 2 runs one timestep BEHIND layer 1 (software pipeline) so the two
    recurrence chains overlap.  Its v2 multiply runs on GPSIMD to keep the
    DVE queue tight.  (All elementwise operands share base partition 0 --
    the neuronxcc verifier rejects split-base accesses.)
  - Layer-1 input+bias matmuls for step t+1 are issued during step t into the
    other PSUM buffer (bufs=2), so only the 4 hidden matmuls precede the gate
    activation on the critical path.
"""

import os
import numpy as np

import concourse.bass as bass
import concourse.mybir as mybir
from concourse import bacc
from concourse.tile import TileContext
from concourse.bass_utils import run_bass_kernel_spmd

N_CORES = 8
B, T, D = 2048, 128, 128
H1, H2, OUT = 128, 64, 64
BC = B // N_CORES  # 256 batch per core
X_CHUNKS = [(0, 4), (4, 16), (16, 48), (48, 128)]  # staged x DMA (ramp-friendly)

FP = mybir.dt.float32
F16 = mybir.dt.float16
AF = mybir.ActivationFunctionType
ALU = mybir.AluOpType

# packed-weight column offsets: one DMA loads every constant (9 separate
# DMAs cost ~1.2us of DGE fixed overhead each on the startup ramp)
OFF_W1, OFF_B1 = 0, 512
OFF_ONES, OFF_WZ = 1024, 1280
WCRIT = 1408  # end of the step-0-critical slice (first DMA)
OFF_WH1, OFF_W2 = 1408, 1920
OFF_WH2, OFF_B2 = 2176, 2432
OFF_WD, OFF_BD = 2688, 2752
WCOLS = 2816

_PROGRAM_CACHE = {}


def build_program():
    if "nc" in _PROGRAM_CACHE:
        return _PROGRAM_CACHE["nc"]

    nc = bacc.Bacc(
        "TRN2", target_bir_lowering=False, debug=False,
        enable_asserts=False, num_devices=N_CORES,
    )

    # ---- DRAM parameters (per-core shapes; in_maps supply per-core data)
    xT_d = nc.declare_dram_parameter("xT", [D, T, BC], F16, isOutput=False)
    wpack_d = nc.declare_dram_parameter("wpack", [D, WCOLS], F16, isOutput=False)
    out_d = nc.declare_dram_parameter("outT", [OUT, BC], FP, isOutput=True)

    with TileContext(nc, num_cores=N_CORES) as tc:
        with (
            tc.tile_pool(name="const", bufs=1) as cpool,
            tc.tile_pool(name="actsbig", bufs=6) as bpool,
            tc.tile_pool(name="acts", bufs=20) as apool,
            tc.tile_pool(name="state", bufs=6) as spool,
            tc.tile_pool(name="ps1a", bufs=2, space="PSUM") as ps1apool,
            tc.tile_pool(name="ps1b", bufs=2, space="PSUM") as ps1bpool,
            tc.tile_pool(name="ps2a", bufs=2, space="PSUM") as ps2apool,
            tc.tile_pool(name="ps2b", bufs=2, space="PSUM") as ps2bpool,
        ):
            # ---- load all constants / weights with ONE DMA
            wpack = cpool.tile([D, WCOLS], F16, tag="wpack")
            nc.sync.dma_start(out=wpack[:, 0:WCRIT], in_=wpack_d[:, 0:WCRIT])
            w1c = lambda j: wpack[:, OFF_W1 + j * H1:OFF_W1 + (j + 1) * H1]
            wh1c = lambda j: wpack[:, OFF_WH1 + j * H1:OFF_WH1 + (j + 1) * H1]
            b1c = lambda j: wpack[0:1, OFF_B1 + j * H1:OFF_B1 + (j + 1) * H1]
            w2c = lambda k: wpack[:, OFF_W2 + k * 2 * H2:OFF_W2 + (k + 1) * 2 * H2]
            wh2c = lambda k: wpack[0:H2, OFF_WH2 + k * 2 * H2:OFF_WH2 + (k + 1) * 2 * H2]
            b2c = lambda k: wpack[0:1, OFF_B2 + k * 2 * H2:OFF_B2 + (k + 1) * 2 * H2]
            wdA = wpack[0:H2, OFF_WD:OFF_WD + OUT]
            bdA = wpack[0:1, OFF_BD:OFF_BD + OUT]
            onesA = wpack[0:1, OFF_ONES:OFF_ONES + BC]
            wzeroA = wpack[0:1, OFF_WZ:OFF_WZ + 2 * H2]

            xs = cpool.tile([D, T, BC], F16, tag="xs")
            a0, b0 = X_CHUNKS[0]
            nc.sync.dma_start(out=xs[:, a0:b0, :], in_=xT_d[:, a0:b0, :])
            nc.sync.dma_start(out=wpack[:, WCRIT:], in_=wpack_d[:, WCRIT:])
            for a, b_ in X_CHUNKS[1:]:
                nc.sync.dma_start(out=xs[:, a:b_, :], in_=xT_d[:, a:b_, :])

            czero = cpool.tile([H1, BC], F16, tag="czero")
            nc.vector.memset(czero[:], 0.0)

            h1p = c1p = h2p = c2p = None  # previous-step states

            def l1_prefetch(t):
                """input+bias matmuls for L1 step t into two fresh PSUM tiles:
                pa = (i, 2g), pb = (f, o). Separate tiles keep the two gate
                ACT halves fully decoupled in the scheduler."""
                pa = ps1apool.tile([H1, 2, BC], FP, tag="p1a")
                pb = ps1bpool.tile([H1, 2, BC], FP, tag="p1b")
                xt = xs[:, t, :]
                last = t == 0  # no hidden matmuls at t=0 -> stops live here
                for j, p in ((0, pa), (1, pa), (2, pb), (3, pb)):
                    nc.tensor.matmul(p[:, j % 2, :], w1c(j),
                                     xt, start=(j in (0, 2)), stop=False)
                for j, p in ((0, pa), (1, pa), (2, pb), (3, pb)):
                    nc.tensor.matmul(p[:, j % 2, :], b1c(j),
                                     onesA, start=False,
                                     stop=(last and j in (1, 3)))
                return pa, pb

            def l2_matmuls(s, h1s, dep_ap=None):
                """all matmuls for L2 step s (input from h1s, hidden from h2p).
                Gates packed pairwise on 128 partitions across two separate
                PSUM tiles: pa=[f|2g], pb=[i|o].  Each tile gets its own
                closing zero-weight matmul folding in dep_ap (if given) as a
                pure scheduling dependency, so each L2 gate-ACT half waits
                only its own column and never precedes the first L1 half."""
                pa = ps2apool.tile([2 * H2, BC], FP, tag="p2a")
                pb = ps2bpool.tile([2 * H2, BC], FP, tag="p2b")
                gate = dep_ap is not None
                for k, p in ((0, pa), (1, pb)):
                    nc.tensor.matmul(p[:], w2c(k), h1s[:],
                                     start=True, stop=False)
                for k, p in ((0, pa), (1, pb)):
                    nc.tensor.matmul(p[:], b2c(k), onesA,
                                     start=False, stop=(not gate and s == 0))
                if s > 0:
                    for k, p in ((0, pa), (1, pb)):
                        nc.tensor.matmul(p[:], wh2c(k), h2p[:],
                                         start=False, stop=(not gate))
                if gate:
                    nc.tensor.matmul(pa[:], wzeroA, dep_ap, start=False, stop=True)
                    nc.tensor.matmul(pb[:], wzeroA, dep_ap, start=False, stop=True)
                return pa, pb

            p1a, p1b = l1_prefetch(0)
            for t in range(T + 1):
                s = t - 1  # L2 step handled this iteration
                if t < T:
                    # -- PE: L1 hidden matmuls for step t (chain-critical).
                    # Gate order (i, 2g, f, o): the (i,g) tile closes after two
                    # matmuls so its gate-ACT half starts early; both of u's
                    # inputs are in that half, so the gt/u subchain overlaps
                    # the second half.
                    if t > 0:
                        for j, p in ((0, p1a), (1, p1a), (2, p1b), (3, p1b)):
                            nc.tensor.matmul(p[:, j % 2, :], wh1c(j),
                                             h1p[:], start=False, stop=(j in (1, 3)))

                    # -- ACT: L1 gates in two halves (i,g) then (f,o)
                    sga = bpool.tile([H1, 2, BC], F16, tag="sga")
                    sgb = bpool.tile([H1, 2, BC], F16, tag="sgb")
                    nc.scalar.activation(sga[:], p1a[:], AF.Sigmoid)
                    nc.scalar.activation(sgb[:], p1b[:], AF.Sigmoid)

                # -- PE: all matmuls for L2 step s (off-chain)
                if s >= 0:
                    p2a, p2b = l2_matmuls(s, h1p,
                                          dep_ap=sga[0:1, 0, :] if t < T else None)

                if t < T:
                    # -- DVE: L1 cell update (gt = tanh(g) = 2*sg_g - 1)
                    gt = apool.tile([H1, BC], F16, tag="gt")
                    v = apool.tile([H1, BC], F16, tag="v")
                    u = apool.tile([H1, BC], F16, tag="u")
                    c1n = spool.tile([H1, BC], F16, tag="c1")
                    nc.vector.tensor_scalar(gt[:], sga[:, 1, :], 2.0, 1.0,
                                            op0=ALU.mult, op1=ALU.subtract)
                    nc.vector.tensor_tensor(u[:], sga[:, 0, :], gt[:], op=ALU.mult)
                    nc.vector.tensor_tensor(v[:], sgb[:, 0, :],
                                            czero[:] if t == 0 else c1p[:], op=ALU.mult)
                    nc.vector.tensor_tensor(c1n[:], u[:], v[:], op=ALU.add)

                if s >= 0:
                    # -- ACT: L2 gates in two packed halves: a=[f|2g], b=[i|o].
                    # The (f,2g) half feeds the long poles (Pool v2, gt2) first.
                    sg2a = apool.tile([2 * H2, BC], F16, tag="sg2a")
                    sg2b = apool.tile([2 * H2, BC], F16, tag="sg2b")
                    nc.scalar.activation(sg2a[:], p2a[:], AF.Sigmoid)
                    nc.scalar.activation(sg2b[:], p2b[:], AF.Sigmoid)
                    # -- POOL: v2 (off the DVE queue; operands at base 0)
                    v2 = apool.tile([H2, BC], F16, tag="v2")
                    nc.vector.tensor_tensor(v2[:], sg2a[0:H2, :],
                                            czero[0:H2, :] if s == 0 else c2p[:],
                                            op=ALU.mult)
                    # -- DVE: gt2/so2 shift partitions 64-127 down to base 0
                    # (single-input ops may shift; two-input ops may not)
                    gt2 = apool.tile([H2, BC], F16, tag="gt2")
                    so2 = apool.tile([H2, BC], F16, tag="so2")
                    u2 = apool.tile([H2, BC], F16, tag="u2")
                    nc.vector.tensor_scalar(gt2[:], sg2a[H2:2 * H2, :], 2.0, 1.0,
                                            op0=ALU.mult, op1=ALU.subtract)
                    nc.vector.tensor_scalar(so2[:], sg2b[H2:2 * H2, :], 1.0, 0.0,
                                            op0=ALU.mult, op1=ALU.add)
                    nc.vector.tensor_tensor(u2[:], sg2b[0:H2, :], gt2[:], op=ALU.mult)

                if t < T:
                    # -- ACT: thc1 (chain), then DVE: h1n (chain)
                    thc1 = apool.tile([H1, BC], F16, tag="thc1")
                    nc.scalar.activation(thc1[:], c1n[:], AF.Tanh)
                    h1n = spool.tile([H1, BC], F16, tag="h1")
                    nc.vector.tensor_tensor(h1n[:], sgb[:, 1, :], thc1[:], op=ALU.mult)

                if s >= 0:
                    # -- DVE: c2n; ACT: thc2; DVE: h2n
                    c2n = spool.tile([H2, BC], F16, tag="c2")
                    nc.vector.tensor_tensor(c2n[:], u2[:], v2[:], op=ALU.add)
                    thc2 = apool.tile([H2, BC], F16, tag="thc2")
                    nc.scalar.activation(thc2[:], c2n[:], AF.Tanh)
                    h2n = spool.tile([H2, BC], F16, tag="h2")
                    nc.vector.tensor_tensor(h2n[:], so2[:], thc2[:], op=ALU.mult)
                    h2p, c2p = h2n, c2n

                # -- PE: prefetch L1 input+bias for step t+1
                if t < T - 1:
                    p1_next = l1_prefetch(t + 1)

                if t < T:
                    c1p, h1p = c1n, h1n
                if t < T - 1:
                    p1a, p1b = p1_next

            # ---- dense head on h2[T-1]
            pd = ps2apool.tile([OUT, BC], FP, tag="p2a")
            nc.tensor.matmul(pd[:], wdA, h2p[:], start=True, stop=False)
            nc.tensor.matmul(pd[:], bdA, onesA, start=False, stop=True)
            outs = cpool.tile([OUT, BC], FP, tag="outs")
            nc.scalar.activation(outs[:], pd[:], AF.Relu)
            nc.sync.dma_start(out=out_d[:], in_=outs[:])

    nc.finalize()
    _PROGRAM_CACHE["nc"] = nc
    return nc


def _prep_inputs(x, w_ih1, w_hh1, b_ih1, b_hh1, w_ih2, w_hh2, b_ih2, b_hh2,
                 w_dense, b_dense):
    """Host-side layout prep (fp16). Device gate order: [i, f, o, 2g] for L1;
    packed [i | 2g], [f | o] columns for L2. g weights doubled because
    tanh(z) = 2*sigmoid(2z) - 1 on device."""
    f16 = np.float16

    def gates(w_t, H):  # w_t: [in, 4H] torch order (i,f,g,o)
        i, f, g, o = (np.float64(w_t[:, k * H:(k + 1) * H]) for k in range(4))
        return i, f, 2.0 * g, o

    def cat(parts):
        return np.concatenate(parts, axis=-1).astype(f16)

    i1, f1, g1, o1 = gates(w_ih1.T, H1)
    w1 = cat([i1, g1, f1, o1])
    i1, f1, g1, o1 = gates(w_hh1.T, H1)
    wh1 = cat([i1, g1, f1, o1])
    i1, f1, g1, o1 = gates((b_ih1 + b_hh1)[None, :], H1)
    b1 = cat([i1, g1, f1, o1])

    i2, f2, g2, o2 = gates(w_ih2.T, H2)
    w2 = cat([f2, g2, i2, o2])
    i2, f2, g2, o2 = gates(w_hh2.T, H2)
    wh2 = cat([f2, g2, i2, o2])
    i2, f2, g2, o2 = gates((b_ih2 + b_hh2)[None, :], H2)
    b2 = cat([f2, g2, i2, o2])

    wd = np.float64(w_dense.T).astype(f16)
    bd = b_dense.astype(f16)[None, :]

    wpack = np.zeros((D, WCOLS), f16)
    wpack[:, OFF_W1:OFF_W1 + 4 * H1] = w1
    wpack[:, OFF_WH1:OFF_WH1 + 4 * H1] = wh1
    wpack[:, OFF_W2:OFF_W2 + 4 * H2] = w2
    wpack[0:H2, OFF_WH2:OFF_WH2 + 4 * H2] = wh2
    wpack[0:1, OFF_B1:OFF_B1 + 4 * H1] = b1
    wpack[0:1, OFF_B2:OFF_B2 + 4 * H2] = b2
    wpack[0:H2, OFF_WD:OFF_WD + OUT] = wd
    wpack[0:1, OFF_BD:OFF_BD + OUT] = bd
    wpack[0:1, OFF_ONES:OFF_ONES + BC] = 1.0

    xT = np.asarray(x, dtype=f16).transpose(2, 1, 0)  # [D,T,B]
    shared = dict(wpack=wpack)
    in_maps = []
    for c in range(N_CORES):
        m = dict(shared)
        m["xT"] = np.ascontiguousarray(xT[:, :, c * BC:(c + 1) * BC])
        in_maps.append(m)
    return in_maps


def _run(inputs, trace=False, **kw):
    nc = build_program()
    in_maps = _prep_inputs(**inputs)
    res = run_bass_kernel_spmd(nc, in_maps, list(range(N_CORES)), trace=trace, **kw)
    out = np.concatenate([np.asarray(res.results[c]["outT"]).T for c in range(N_CORES)], axis=0)
    return out.astype(np.float32), res


def kernel(**inputs):
    out, _ = _run(inputs, trace=False)
    return out


if __name__ == "__main__":
    import reference
    inputs = {k: np.asarray(v) for k, v in reference.setup_inputs().items()}
    expected = np.asarray(reference.reference(**inputs))
    out, res = _run(inputs, trace=os.environ.get("KTRACE", "0") == "1")
    err = np.abs(out - expected)
    rel = err.max() / (np.abs(expected).max() + 1e-12)
    print("max abs err:", err.max(), "rel:", rel)
    print("exec_time_ns:", res.exec_time_ns)
